# revision 1
# baseline (speedup 1.0000x reference)
"""Bilateral-grid slice v2: bf16 table (256B rows), bf16 blend with tree-adds,
ACT-engine offload, mod-based floor, direct DVE idx copies, tunable gather
call size / queue count.

Self-contained: expects full unsharded inputs, returns the full output.
"""
from contextlib import ExitStack

import numpy as np

import concourse.bass as bass
import concourse.bacc as bacc
import concourse.tile as tile
import concourse.mybir as mybir

F32 = mybir.dt.float32
BF16 = mybir.dt.float16  # fp16: 8x finer mantissa than bf16, same 2-byte DVE speedups
I16 = mybir.dt.int16
AL = mybir.AluOpType
AX = mybir.AxisListType
ACTF = mybir.ActivationFunctionType

GRID_CELLS = 2048  # 8z * 16y * 16x
GRID_PAD = 2048 + 384
CORNER_OFF = [dz * 256 + dy * 16 + dx for dz in (0, 1) for dy in (0, 1) for dx in (0, 1)]

N_CORES = 8
GRAY_W = (0.299, 0.587, 0.114)


def _mkap(t_ap, p0, pn, free, off_elems):
    pitch = t_ap.ap[0][0]
    return bass.AP(
        tensor=t_ap.tensor,
        offset=t_ap.offset + p0 * pitch + off_elems,
        ap=[[pitch, pn]] + [[s, c] for (s, c) in free],
    )


def _bsel_matrices():
    b = np.zeros((128, 8, 128), dtype=np.float32)
    for h in range(8):
        for m in range(128):
            b[16 * h + (m % 16), h, m] = 1.0
    return b


def _build_core_kernel(n_views, Npx, T=512, COLS=4, MERGE=16, NQ=4,
                       scratch=49152, num_devices=8,
                       no_blend=False, no_gather=False, psf_bufs=4, gat_bufs=4):
    """COLS: gather-call size in queue-columns (ipc = COLS*128).
    MERGE: blend batch in queue-columns (must be multiple of COLS or vice versa)."""
    assert Npx % 128 == 0
    B = Npx // 128
    nT = (B + T - 1) // T
    nCC = GRID_CELLS // 128

    nc = bacc.Bacc("TRN2", debug=False, num_devices=num_devices,
                   dynamic_dma_scratch_size=scratch, num_swdge_queues=NQ)
    grids_d = nc.dram_tensor("grids", [n_views, 12, GRID_CELLS], F32, kind="ExternalInput").ap()
    gxy_d = nc.dram_tensor("gxy", [n_views, Npx, 2], BF16, kind="ExternalInput").ap()
    rgb_d = nc.dram_tensor("rgb", [n_views, Npx, 3], BF16, kind="ExternalInput").ap()
    bsel_d = nc.dram_tensor("bsel", [128, 8, 128], F32, kind="ExternalInput").ap()
    out_d = nc.dram_tensor("out", [n_views, Npx, 3], BF16, kind="ExternalOutput").ap()

    JQmax = (T + NQ - 1) // NQ

    with tile.TileContext(nc) as tc, ExitStack() as ctx:
        consts = ctx.enter_context(tc.tile_pool(name="consts", bufs=1))
        gridp = ctx.enter_context(tc.tile_pool(name="gridp", bufs=1))
        cubep = ctx.enter_context(tc.tile_pool(name="cubep", bufs=2))
        dramp = ctx.enter_context(tc.tile_pool(name="dramp", bufs=2, space="DRAM"))
        psT = ctx.enter_context(tc.tile_pool(name="psT", bufs=4, space="PSUM"))
        psF = ctx.enter_context(tc.tile_pool(name="psF", bufs=psf_bufs, space="PSUM"))
        inp = ctx.enter_context(tc.tile_pool(name="inp", bufs=3))
        scal = ctx.enter_context(tc.tile_pool(name="scal", bufs=1))
        wp = ctx.enter_context(tc.tile_pool(name="wp", bufs=1))
        idxp = ctx.enter_context(tc.tile_pool(name="idxp", bufs=3))
        gat = ctx.enter_context(tc.tile_pool(name="gat", bufs=gat_bufs))
        bln = ctx.enter_context(tc.tile_pool(name="bln", bufs=1))
        outp = ctx.enter_context(tc.tile_pool(name="outp", bufs=3))

        ident = consts.tile([128, 128], F32)
        from concourse.masks import make_identity
        make_identity(nc, ident)
        bsel_sb = consts.tile([128, 8, 128], F32)
        nc.sync.dma_start(out=bsel_sb, in_=bsel_d)

        for v in range(n_views):
            # ---- cube table (bf16, 256B rows) ----
            grid_sb = gridp.tile([12, GRID_PAD], F32, tag="grid", name="grid")
            nc.vector.memset(grid_sb[:, GRID_CELLS:], 0.0)
            nc.sync.dma_start(out=grid_sb[:, :GRID_CELLS], in_=grids_d[v])
            cube_sb = cubep.tile([128, nCC, 128], BF16, tag="cube", name="cube")
            nc.vector.memset(cube_sb[:, :, 96:], 0.0)
            for m in range(nCC):
                for jj in range(8):
                    ps = psT.tile([128, 12], F32, tag="pst", name="pst")
                    nc.tensor.transpose(
                        ps[:],
                        grid_sb[:, m * 128 + CORNER_OFF[jj]: m * 128 + CORNER_OFF[jj] + 128],
                        ident[:12, :12])
                    nc.vector.tensor_copy(
                        _mkap(cube_sb[:], 0, 128, [(8, 12)], m * 128 + jj), ps[:])
            cube_dram = dramp.tile([GRID_CELLS, 128], BF16, tag="ctab", name="ctab")
            for m in range(nCC):
                nc.sync.dma_start(out=cube_dram[m * 128:(m + 1) * 128, :],
                                  in_=cube_sb[:, m, :])

            # ---- pixel chunks ----
            for ci in range(nT):
                t0 = ci * T
                Tc = min(T, B - t0)
                gxy_t = inp.tile([128, T, 2], BF16, tag="gxy", name="gxyt")
                rgb_t = inp.tile([128, T, 3], BF16, tag="rgb", name="rgbt")
                gxy_v = gxy_d[v].rearrange("(p b) k -> p b k", p=128)
                rgb_v = rgb_d[v].rearrange("(p b) k -> p b k", p=128)
                nc.sync.dma_start(out=gxy_t[:, :Tc], in_=gxy_v[:, t0:t0 + Tc])
                nc.sync.dma_start(out=rgb_t[:, :Tc], in_=rgb_v[:, t0:t0 + Tc])

                def st(tag, dt=F32):
                    return scal.tile([128, T], dt, tag=tag, name=tag)

                # coordinates (ACT for the scaling ops)
                x_t, y_t, z_t = st("x"), st("y"), st("z")
                nc.scalar.activation(x_t[:, :Tc], gxy_t[:, :Tc, 0], ACTF.Copy, scale=15.0)
                nc.scalar.activation(y_t[:, :Tc], gxy_t[:, :Tc, 1], ACTF.Copy, scale=15.0)
                nc.scalar.activation(z_t[:, :Tc], rgb_t[:, :Tc, 0], ACTF.Copy,
                                     scale=GRAY_W[0] * 7)
                nc.vector.scalar_tensor_tensor(
                    z_t[:, :Tc], rgb_t[:, :Tc, 1], GRAY_W[1] * 7, z_t[:, :Tc], AL.mult, AL.add)
                nc.vector.scalar_tensor_tensor(
                    z_t[:, :Tc], rgb_t[:, :Tc, 2], GRAY_W[2] * 7, z_t[:, :Tc], AL.mult, AL.add)

                # floor: round-to-int then fix up, clamp, frac
                f_t, c0_t = {}, {}
                ii_t = scal.tile([128, T], mybir.dt.int32, tag="ii", name="ii")
                for nm, src, hi in (("x", x_t, 14.0), ("y", y_t, 14.0), ("z", z_t, 6.0)):
                    nc.vector.tensor_copy(ii_t[:, :Tc], src[:, :Tc])
                    c0 = st(nm + "0")
                    nc.vector.tensor_copy(c0[:, :Tc], ii_t[:, :Tc])
                    fr = st("fr")
                    nc.vector.tensor_tensor(fr[:, :Tc], c0[:, :Tc], src[:, :Tc], AL.is_gt)
                    nc.vector.tensor_tensor(c0[:, :Tc], c0[:, :Tc], fr[:, :Tc], AL.subtract)
                    nc.vector.tensor_scalar_min(c0[:, :Tc], c0[:, :Tc], hi)
                    f = st("f" + nm, BF16)
                    nc.vector.tensor_tensor(f[:, :Tc], src[:, :Tc], c0[:, :Tc], AL.subtract)
                    f_t[nm], c0_t[nm] = f, c0

                cellf = st("cellf")
                nc.vector.scalar_tensor_tensor(
                    cellf[:, :Tc], c0_t["z"][:, :Tc], 16.0, c0_t["y"][:, :Tc], AL.mult, AL.add)
                nc.vector.scalar_tensor_tensor(
                    cellf[:, :Tc], cellf[:, :Tc], 16.0, c0_t["x"][:, :Tc], AL.mult, AL.add)

                # 1-f on ACT (bf16 out)
                omx, omy, omz = st("omx", BF16), st("omy", BF16), st("omz", BF16)
                nc.scalar.activation(omx[:, :Tc], f_t["x"][:, :Tc], ACTF.Copy,
                                     scale=-1.0, bias=1.0)
                nc.scalar.activation(omy[:, :Tc], f_t["y"][:, :Tc], ACTF.Copy,
                                     scale=-1.0, bias=1.0)
                nc.scalar.activation(omz[:, :Tc], f_t["z"][:, :Tc], ACTF.Copy,
                                     scale=-1.0, bias=1.0)

                pyx = []
                for wy in (omy, f_t["y"]):
                    for wx in (omx, f_t["x"]):
                        p = scal.tile([128, T], BF16, tag=f"pyx{len(pyx)}",
                                      name=f"pyx{len(pyx)}")
                        nc.vector.tensor_tensor(p[:, :Tc], wy[:, :Tc], wx[:, :Tc], AL.mult)
                        pyx.append(p)
                w8_t = wp.tile([128, T, 8], BF16, tag="w8", name="w8")
                for jj in range(8):
                    wz = omz if jj < 4 else f_t["z"]
                    nc.vector.tensor_tensor(
                        w8_t[:, :Tc, jj], wz[:, :Tc], pyx[jj % 4][:, :Tc], AL.mult)

                out_t = outp.tile([128, T, 3], BF16, tag="out", name="outt")

                # idx build: PE bsel matmuls + strided PSUM->SBUF copies
                idx_all = idxp.tile([128, JQmax, 8], I16, tag="idx", name="idxall")
                JQ = [len(range(q, Tc, NQ)) for q in range(NQ)]
                for h in range(8):
                    pf = psF.tile([128, T], F32, tag="psf", name="psf")
                    nc.tensor.matmul(pf[:, :Tc], bsel_sb[:, h, :], cellf[:, :Tc],
                                     start=True, stop=True)
                    for q in range(NQ):
                        if JQ[q]:
                            nc.vector.tensor_copy(
                                _mkap(idx_all[:], 32 * q, 32, [(8, JQ[q])], h),
                                _mkap(pf[:], 32 * q, 32, [(NQ, JQ[q])], q))

                # Gathers issued round-robin across queues: consecutive calls
                # on the SAME queue serialize DGE against the prior call's
                # transfer (~3x slowdown, HW-measured), so interleave queues.
                cube_q = {}
                blend_jobs = []
                nmb = (max(JQ) + COLS - 1) // COLS if max(JQ) else 0
                for mb in range(nmb):
                    for q in range(NQ):
                        sc = mb * COLS
                        if sc >= JQ[q]:
                            continue
                        m0 = (sc // MERGE) * MERGE
                        if sc == m0:
                            cube_q[q] = gat.tile([128, MERGE, 128], BF16,
                                                 tag=f"cm{q}", name=f"cm{q}")
                        cols = min(COLS, JQ[q] - sc)
                        if not no_gather:
                            nc.gpsimd.dma_gather(
                                out_ap=cube_q[q][:, sc - m0: sc - m0 + cols, :],
                                in_ap=cube_dram[:],
                                idxs_ap=_mkap(idx_all[:], 0, 128, [(1, cols * 8)], sc * 8),
                                num_idxs=cols * 128,
                                num_idxs_reg=cols * 128,
                                elem_size=128,
                                queue_num=q,
                            )
                        if sc + cols >= min(m0 + MERGE, JQ[q]):
                            blend_jobs.append((q, m0, min(MERGE, JQ[q] - m0), cube_q[q]))
                for q, m0, mc, cube_m in (blend_jobs if not no_blend else []):
                        # blend: S = cube * w8 (fp16 2x), then tree-add over j
                        S_t = bln.tile([128, MERGE, 12, 8], BF16, tag="S", name="St")
                        nc.vector.tensor_tensor(
                            S_t[:, :mc],
                            cube_m[:, :mc, :96].rearrange("p m (c j) -> p m c j", j=8),
                            _mkap(w8_t[:], 0, 128, [(NQ * 8, mc), (0, 12), (1, 8)],
                                  (q + NQ * m0) * 8),
                            AL.mult)
                        t1 = bln.tile([128, MERGE, 12, 4], BF16, tag="t1", name="t1")
                        nc.vector.tensor_tensor(
                            t1[:, :mc], S_t[:, :mc, :, 0:4], S_t[:, :mc, :, 4:8], AL.add)
                        t2 = bln.tile([128, MERGE, 12, 2], BF16, tag="t2", name="t2")
                        nc.vector.tensor_tensor(
                            t2[:, :mc], t1[:, :mc, :, 0:2], t1[:, :mc, :, 2:4], AL.add)
                        aff = bln.tile([128, MERGE, 12], BF16, tag="aff", name="aff")
                        nc.vector.tensor_tensor(
                            aff[:, :mc],
                            _mkap(t2[:], 0, 128, [(24, mc), (2, 12)], 0),
                            _mkap(t2[:], 0, 128, [(24, mc), (2, 12)], 1),
                            AL.add)
                        # affine: S2 = aff[:, :, i, c]*rgb (bf16 2x), adds
                        S2 = bln.tile([128, MERGE, 3, 3], BF16, tag="S2", name="S2")
                        nc.vector.tensor_tensor(
                            S2[:, :mc],
                            _mkap(aff[:], 0, 128, [(12, mc), (4, 3), (1, 3)], 0),
                            _mkap(rgb_t[:], 0, 128, [(NQ * 3, mc), (0, 3), (1, 3)],
                                  (q + NQ * m0) * 3),
                            AL.mult)
                        a1 = bln.tile([128, MERGE, 3], BF16, tag="a1", name="a1")
                        nc.vector.tensor_tensor(
                            a1[:, :mc],
                            _mkap(S2[:], 0, 128, [(9, mc), (3, 3)], 0),
                            _mkap(S2[:], 0, 128, [(9, mc), (3, 3)], 1),
                            AL.add)
                        a2 = bln.tile([128, MERGE, 3], BF16, tag="a2", name="a2")
                        nc.vector.tensor_tensor(
                            a2[:, :mc],
                            _mkap(S2[:], 0, 128, [(9, mc), (3, 3)], 2),
                            _mkap(aff[:], 0, 128, [(12, mc), (4, 3)], 3),
                            AL.add)
                        nc.vector.tensor_tensor(
                            _mkap(out_t[:], 0, 128, [(NQ * 3, mc), (1, 3)], (q + NQ * m0) * 3),
                            a1[:, :mc], a2[:, :mc], AL.add)

                if no_blend:
                    nc.vector.tensor_copy(out_t[:, :Tc], rgb_t[:, :Tc])
                out_v = out_d[v].rearrange("(p b) k -> p b k", p=128)
                nc.sync.dma_start(out=out_v[:, t0:t0 + Tc], in_=out_t[:, :Tc])

    nc.finalize()
    return nc


# ---------------- PJRT runner ----------------

def _make_runner(nc, n_cores):
    import jax
    import jax.core
    from jax.sharding import Mesh, PartitionSpec
    from jax.experimental.shard_map import shard_map
    from concourse.bass2jax import _bass_exec_p, partition_id_tensor, install_neuronx_cc_hook

    install_neuronx_cc_hook()
    partition_name = nc.partition_id_tensor.name if nc.partition_id_tensor else None
    in_names, out_names, out_avals, zero_shapes = [], [], [], []
    for alloc in nc.m.functions[0].allocations:
        if not isinstance(alloc, mybir.MemoryLocationSet):
            continue
        name = alloc.memorylocations[0].name
        if alloc.kind == "ExternalInput":
            if name != partition_name:
                in_names.append(name)
        elif alloc.kind == "ExternalOutput":
            shape = tuple(alloc.tensor_shape)
            dtype = mybir.dt.np(alloc.dtype)
            out_names.append(name)
            out_avals.append(jax.core.ShapedArray(shape, dtype))
            zero_shapes.append((shape, dtype))

    n_params = len(in_names)
    n_outs = len(out_avals)
    all_in_names = list(in_names) + list(out_names)
    if partition_name is not None:
        all_in_names.append(partition_name)

    def _body(*args):
        operands = list(args)
        if partition_name is not None:
            operands.append(partition_id_tensor())
        return tuple(_bass_exec_p.bind(
            *operands,
            out_avals=tuple(out_avals),
            in_names=tuple(all_in_names),
            out_names=tuple(out_names),
            lowering_input_output_aliases=(),
            sim_require_finite=False,
            sim_require_nnan=False,
            nc=nc,
        ))

    donate = tuple(range(n_params, n_params + n_outs))
    devices = jax.devices()[:n_cores]
    mesh = Mesh(np.asarray(devices), ("core",))
    in_specs = (PartitionSpec("core"),) * (n_params + n_outs)
    out_specs = (PartitionSpec("core"),) * n_outs
    jf = jax.jit(
        shard_map(_body, mesh=mesh, in_specs=in_specs, out_specs=out_specs,
                  check_rep=False),
        donate_argnums=donate, keep_unused=True)

    def run(in_maps):
        concat_in = [
            np.concatenate([np.asarray(in_maps[c][n]) for c in range(n_cores)], axis=0)
            for n in in_names
        ]
        concat_zeros = [np.zeros((n_cores * s[0], *s[1:]), d) for (s, d) in zero_shapes]
        out_arrs = jf(*concat_in, *concat_zeros)
        jax.block_until_ready(out_arrs)
        return [
            {n: np.asarray(out_arrs[i]).reshape(n_cores, *out_avals[i].shape)[c]
             for i, n in enumerate(out_names)}
            for c in range(n_cores)
        ]

    return run, jf, in_names, zero_shapes


_CACHE = {}


def _get_runner(n_views, Npx):
    key = (n_views, Npx)
    if key not in _CACHE:
        nc = _build_core_kernel(n_views, Npx)
        _CACHE[key] = _make_runner(nc, N_CORES)
    return _CACHE[key]


def kernel(grids, grid_xy, rgb, idx):
    grids = np.ascontiguousarray(np.asarray(grids, dtype=np.float32))
    grid_xy = np.asarray(grid_xy, dtype=np.float32).astype(np.float16)
    rgb = np.asarray(rgb, dtype=np.float32).astype(np.float16)
    idx = np.asarray(idx)
    N, Hi, Wi, _ = rgb.shape
    Npx = Hi * Wi
    V = N // N_CORES
    g = np.take(grids, idx, axis=0)  # (N, 12, 8, 16, 16)

    run, _, _, _ = _get_runner(V, Npx)
    bsel = _bsel_matrices()
    in_maps = [{
        "grids": g[c * V:(c + 1) * V].reshape(V, 12, -1),
        "gxy": grid_xy[c * V:(c + 1) * V].reshape(V, Npx, 2),
        "rgb": rgb[c * V:(c + 1) * V].reshape(V, Npx, 3),
        "bsel": bsel,
    } for c in range(N_CORES)]
    results = run(in_maps)
    out = np.stack([r["out"] for r in results], axis=0)  # (8, V, Npx, 3) bf16
    return out.reshape(N, Hi, Wi, 3).astype(np.float32)



# revision 2
# speedup vs baseline: 1.7781x; 1.7781x over previous
"""Bilateral-grid slice v2: bf16 table (256B rows), bf16 blend with tree-adds,
ACT-engine offload, mod-based floor, direct DVE idx copies, tunable gather
call size / queue count.

Self-contained: expects full unsharded inputs, returns the full output.
"""
from contextlib import ExitStack

import numpy as np

import concourse.bass as bass
import concourse.bacc as bacc
import concourse.tile as tile
import concourse.mybir as mybir

F32 = mybir.dt.float32
BF16 = mybir.dt.float16  # fp16: 8x finer mantissa than bf16, same 2-byte DVE speedups
I16 = mybir.dt.int16
AL = mybir.AluOpType
AX = mybir.AxisListType
ACTF = mybir.ActivationFunctionType

GRID_CELLS = 2048  # 8z * 16y * 16x
GRID_PAD = 2048 + 384
CORNER_OFF = [dz * 256 + dy * 16 + dx for dz in (0, 1) for dy in (0, 1) for dx in (0, 1)]

N_CORES = 8
GRAY_W = (0.299, 0.587, 0.114)


def _mkap(t_ap, p0, pn, free, off_elems):
    pitch = t_ap.ap[0][0]
    return bass.AP(
        tensor=t_ap.tensor,
        offset=t_ap.offset + p0 * pitch + off_elems,
        ap=[[pitch, pn]] + [[s, c] for (s, c) in free],
    )


def _bsel_matrices():
    b = np.zeros((128, 8, 128), dtype=np.float32)
    for h in range(8):
        for m in range(128):
            b[16 * h + (m % 16), h, m] = 1.0
    return b


def _build_core_kernel(n_views, Npx, T=512, COLS=4, MERGE=16, NQ=4,
                       scratch=49152, num_devices=8,
                       no_blend=False, no_gather=False, psf_bufs=4, gat_bufs=4):
    """COLS: gather-call size in queue-columns (ipc = COLS*128).
    MERGE: blend batch in queue-columns (must be multiple of COLS or vice versa)."""
    assert Npx % 128 == 0
    B = Npx // 128
    nT = (B + T - 1) // T
    nCC = GRID_CELLS // 128

    nc = bacc.Bacc("TRN2", debug=False, num_devices=num_devices,
                   dynamic_dma_scratch_size=scratch, num_swdge_queues=NQ)
    grids_d = nc.dram_tensor("grids", [n_views, 12, GRID_CELLS], F32, kind="ExternalInput").ap()
    gxy_d = nc.dram_tensor("gxy", [n_views, Npx, 2], BF16, kind="ExternalInput").ap()
    rgb_d = nc.dram_tensor("rgb", [n_views, Npx, 3], BF16, kind="ExternalInput").ap()
    bsel_d = nc.dram_tensor("bsel", [128, 8, 128], F32, kind="ExternalInput").ap()
    out_d = nc.dram_tensor("out", [n_views, Npx, 3], BF16, kind="ExternalOutput").ap()

    JQmax = (T + NQ - 1) // NQ

    with tile.TileContext(nc) as tc, ExitStack() as ctx:
        consts = ctx.enter_context(tc.tile_pool(name="consts", bufs=1))
        gridp = ctx.enter_context(tc.tile_pool(name="gridp", bufs=1))
        cubep = ctx.enter_context(tc.tile_pool(name="cubep", bufs=2))
        dramp = ctx.enter_context(tc.tile_pool(name="dramp", bufs=2, space="DRAM"))
        psT = ctx.enter_context(tc.tile_pool(name="psT", bufs=4, space="PSUM"))
        psF = ctx.enter_context(tc.tile_pool(name="psF", bufs=psf_bufs, space="PSUM"))
        inp = ctx.enter_context(tc.tile_pool(name="inp", bufs=3))
        scal = ctx.enter_context(tc.tile_pool(name="scal", bufs=1))
        wp = ctx.enter_context(tc.tile_pool(name="wp", bufs=1))
        idxp = ctx.enter_context(tc.tile_pool(name="idxp", bufs=3))
        gat = ctx.enter_context(tc.tile_pool(name="gat", bufs=gat_bufs))
        bln = ctx.enter_context(tc.tile_pool(name="bln", bufs=1))
        outp = ctx.enter_context(tc.tile_pool(name="outp", bufs=3))

        ident = consts.tile([128, 128], F32)
        from concourse.masks import make_identity
        make_identity(nc, ident)
        bsel_sb = consts.tile([128, 8, 128], F32)
        nc.sync.dma_start(out=bsel_sb, in_=bsel_d)

        for v in range(n_views):
            # ---- cube table (bf16, 256B rows) ----
            grid_sb = gridp.tile([12, GRID_PAD], F32, tag="grid", name="grid")
            nc.vector.memset(grid_sb[:, GRID_CELLS:], 0.0)
            nc.sync.dma_start(out=grid_sb[:, :GRID_CELLS], in_=grids_d[v])
            cube_sb = cubep.tile([128, nCC, 128], BF16, tag="cube", name="cube")
            nc.vector.memset(cube_sb[:, :, 96:], 0.0)
            for m in range(nCC):
                for jj in range(8):
                    ps = psT.tile([128, 12], F32, tag="pst", name="pst")
                    nc.tensor.transpose(
                        ps[:],
                        grid_sb[:, m * 128 + CORNER_OFF[jj]: m * 128 + CORNER_OFF[jj] + 128],
                        ident[:12, :12])
                    nc.vector.tensor_copy(
                        _mkap(cube_sb[:], 0, 128, [(8, 12)], m * 128 + jj), ps[:])
            cube_dram = dramp.tile([GRID_CELLS, 128], BF16, tag="ctab", name="ctab")
            for m in range(nCC):
                nc.sync.dma_start(out=cube_dram[m * 128:(m + 1) * 128, :],
                                  in_=cube_sb[:, m, :])

            # ---- pixel chunks ----
            for ci in range(nT):
                t0 = ci * T
                Tc = min(T, B - t0)
                gxy_t = inp.tile([128, T, 2], BF16, tag="gxy", name="gxyt")
                rgb_t = inp.tile([128, T, 3], BF16, tag="rgb", name="rgbt")
                gxy_v = gxy_d[v].rearrange("(p b) k -> p b k", p=128)
                rgb_v = rgb_d[v].rearrange("(p b) k -> p b k", p=128)
                nc.sync.dma_start(out=gxy_t[:, :Tc], in_=gxy_v[:, t0:t0 + Tc])
                nc.sync.dma_start(out=rgb_t[:, :Tc], in_=rgb_v[:, t0:t0 + Tc])

                def st(tag, dt=F32):
                    return scal.tile([128, T], dt, tag=tag, name=tag)

                # coordinates (ACT for the scaling ops)
                x_t, y_t, z_t = st("x"), st("y"), st("z")
                nc.scalar.activation(x_t[:, :Tc], gxy_t[:, :Tc, 0], ACTF.Copy, scale=15.0)
                nc.scalar.activation(y_t[:, :Tc], gxy_t[:, :Tc, 1], ACTF.Copy, scale=15.0)
                nc.scalar.activation(z_t[:, :Tc], rgb_t[:, :Tc, 0], ACTF.Copy,
                                     scale=GRAY_W[0] * 7)
                nc.vector.scalar_tensor_tensor(
                    z_t[:, :Tc], rgb_t[:, :Tc, 1], GRAY_W[1] * 7, z_t[:, :Tc], AL.mult, AL.add)
                nc.vector.scalar_tensor_tensor(
                    z_t[:, :Tc], rgb_t[:, :Tc, 2], GRAY_W[2] * 7, z_t[:, :Tc], AL.mult, AL.add)

                # floor: round-to-int then fix up, clamp, frac
                f_t, c0_t = {}, {}
                ii_t = scal.tile([128, T], mybir.dt.int32, tag="ii", name="ii")
                for nm, src, hi in (("x", x_t, 14.0), ("y", y_t, 14.0), ("z", z_t, 6.0)):
                    nc.vector.tensor_copy(ii_t[:, :Tc], src[:, :Tc])
                    c0 = st(nm + "0")
                    nc.vector.tensor_copy(c0[:, :Tc], ii_t[:, :Tc])
                    fr = st("fr")
                    nc.vector.tensor_tensor(fr[:, :Tc], c0[:, :Tc], src[:, :Tc], AL.is_gt)
                    nc.vector.tensor_tensor(c0[:, :Tc], c0[:, :Tc], fr[:, :Tc], AL.subtract)
                    nc.vector.tensor_scalar_min(c0[:, :Tc], c0[:, :Tc], hi)
                    f = st("f" + nm, BF16)
                    nc.vector.tensor_tensor(f[:, :Tc], src[:, :Tc], c0[:, :Tc], AL.subtract)
                    f_t[nm], c0_t[nm] = f, c0

                cellf = st("cellf")
                nc.vector.scalar_tensor_tensor(
                    cellf[:, :Tc], c0_t["z"][:, :Tc], 16.0, c0_t["y"][:, :Tc], AL.mult, AL.add)
                nc.vector.scalar_tensor_tensor(
                    cellf[:, :Tc], cellf[:, :Tc], 16.0, c0_t["x"][:, :Tc], AL.mult, AL.add)

                # 1-f on ACT (bf16 out)
                omx, omy, omz = st("omx", BF16), st("omy", BF16), st("omz", BF16)
                nc.scalar.activation(omx[:, :Tc], f_t["x"][:, :Tc], ACTF.Copy,
                                     scale=-1.0, bias=1.0)
                nc.scalar.activation(omy[:, :Tc], f_t["y"][:, :Tc], ACTF.Copy,
                                     scale=-1.0, bias=1.0)
                nc.scalar.activation(omz[:, :Tc], f_t["z"][:, :Tc], ACTF.Copy,
                                     scale=-1.0, bias=1.0)

                pyx = []
                for wy in (omy, f_t["y"]):
                    for wx in (omx, f_t["x"]):
                        p = scal.tile([128, T], BF16, tag=f"pyx{len(pyx)}",
                                      name=f"pyx{len(pyx)}")
                        nc.vector.tensor_tensor(p[:, :Tc], wy[:, :Tc], wx[:, :Tc], AL.mult)
                        pyx.append(p)
                w8_t = wp.tile([128, T, 8], BF16, tag="w8", name="w8")
                for jj in range(8):
                    wz = omz if jj < 4 else f_t["z"]
                    nc.vector.tensor_tensor(
                        w8_t[:, :Tc, jj], wz[:, :Tc], pyx[jj % 4][:, :Tc], AL.mult)

                out_t = outp.tile([128, T, 3], BF16, tag="out", name="outt")

                # idx build: PE bsel matmuls + strided PSUM->SBUF copies
                idx_all = idxp.tile([128, JQmax, 8], I16, tag="idx", name="idxall")
                JQ = [len(range(q, Tc, NQ)) for q in range(NQ)]
                for h in range(8):
                    pf = psF.tile([128, T], F32, tag="psf", name="psf")
                    nc.tensor.matmul(pf[:, :Tc], bsel_sb[:, h, :], cellf[:, :Tc],
                                     start=True, stop=True)
                    for q in range(NQ):
                        if JQ[q]:
                            nc.vector.tensor_copy(
                                _mkap(idx_all[:], 32 * q, 32, [(8, JQ[q])], h),
                                _mkap(pf[:], 32 * q, 32, [(NQ, JQ[q])], q))

                # Gathers issued round-robin across queues: consecutive calls
                # on the SAME queue serialize DGE against the prior call's
                # transfer (~3x slowdown, HW-measured), so interleave queues.
                cube_q = {}
                blend_jobs = []
                nmb = (max(JQ) + COLS - 1) // COLS if max(JQ) else 0
                for mb in range(nmb):
                    for q in range(NQ):
                        sc = mb * COLS
                        if sc >= JQ[q]:
                            continue
                        m0 = (sc // MERGE) * MERGE
                        if sc == m0:
                            cube_q[q] = gat.tile([128, MERGE, 128], BF16,
                                                 tag=f"cm{q}", name=f"cm{q}")
                        cols = min(COLS, JQ[q] - sc)
                        if not no_gather:
                            nc.gpsimd.dma_gather(
                                out_ap=cube_q[q][:, sc - m0: sc - m0 + cols, :],
                                in_ap=cube_dram[:],
                                idxs_ap=_mkap(idx_all[:], 0, 128, [(1, cols * 8)], sc * 8),
                                num_idxs=cols * 128,
                                num_idxs_reg=cols * 128,
                                elem_size=128,
                                queue_num=q,
                            )
                        if sc + cols >= min(m0 + MERGE, JQ[q]):
                            blend_jobs.append((q, m0, min(MERGE, JQ[q] - m0), cube_q[q]))
                for q, m0, mc, cube_m in (blend_jobs if not no_blend else []):
                        # blend: S = cube * w8 (fp16 2x), then tree-add over j
                        S_t = bln.tile([128, MERGE, 12, 8], BF16, tag="S", name="St")
                        nc.vector.tensor_tensor(
                            S_t[:, :mc],
                            cube_m[:, :mc, :96].rearrange("p m (c j) -> p m c j", j=8),
                            _mkap(w8_t[:], 0, 128, [(NQ * 8, mc), (0, 12), (1, 8)],
                                  (q + NQ * m0) * 8),
                            AL.mult)
                        t1 = bln.tile([128, MERGE, 12, 4], BF16, tag="t1", name="t1")
                        nc.vector.tensor_tensor(
                            t1[:, :mc], S_t[:, :mc, :, 0:4], S_t[:, :mc, :, 4:8], AL.add)
                        t2 = bln.tile([128, MERGE, 12, 2], BF16, tag="t2", name="t2")
                        nc.vector.tensor_tensor(
                            t2[:, :mc], t1[:, :mc, :, 0:2], t1[:, :mc, :, 2:4], AL.add)
                        aff = bln.tile([128, MERGE, 12], BF16, tag="aff", name="aff")
                        nc.vector.tensor_tensor(
                            aff[:, :mc],
                            _mkap(t2[:], 0, 128, [(24, mc), (2, 12)], 0),
                            _mkap(t2[:], 0, 128, [(24, mc), (2, 12)], 1),
                            AL.add)
                        # affine: S2 = aff[:, :, i, c]*rgb (bf16 2x), adds
                        S2 = bln.tile([128, MERGE, 3, 3], BF16, tag="S2", name="S2")
                        nc.vector.tensor_tensor(
                            S2[:, :mc],
                            _mkap(aff[:], 0, 128, [(12, mc), (4, 3), (1, 3)], 0),
                            _mkap(rgb_t[:], 0, 128, [(NQ * 3, mc), (0, 3), (1, 3)],
                                  (q + NQ * m0) * 3),
                            AL.mult)
                        a1 = bln.tile([128, MERGE, 3], BF16, tag="a1", name="a1")
                        nc.vector.tensor_tensor(
                            a1[:, :mc],
                            _mkap(S2[:], 0, 128, [(9, mc), (3, 3)], 0),
                            _mkap(S2[:], 0, 128, [(9, mc), (3, 3)], 1),
                            AL.add)
                        a2 = bln.tile([128, MERGE, 3], BF16, tag="a2", name="a2")
                        nc.vector.tensor_tensor(
                            a2[:, :mc],
                            _mkap(S2[:], 0, 128, [(9, mc), (3, 3)], 2),
                            _mkap(aff[:], 0, 128, [(12, mc), (4, 3)], 3),
                            AL.add)
                        nc.vector.tensor_tensor(
                            _mkap(out_t[:], 0, 128, [(NQ * 3, mc), (1, 3)], (q + NQ * m0) * 3),
                            a1[:, :mc], a2[:, :mc], AL.add)

                if no_blend:
                    nc.vector.tensor_copy(out_t[:, :Tc], rgb_t[:, :Tc])
                out_v = out_d[v].rearrange("(p b) k -> p b k", p=128)
                nc.sync.dma_start(out=out_v[:, t0:t0 + Tc], in_=out_t[:, :Tc])

    nc.finalize()
    return nc


# ---------------- PJRT runner ----------------

def _make_runner(nc, n_cores):
    import jax
    import jax.core
    from jax.sharding import Mesh, PartitionSpec
    from jax.experimental.shard_map import shard_map
    from concourse.bass2jax import _bass_exec_p, partition_id_tensor, install_neuronx_cc_hook

    install_neuronx_cc_hook()
    partition_name = nc.partition_id_tensor.name if nc.partition_id_tensor else None
    in_names, out_names, out_avals, zero_shapes = [], [], [], []
    for alloc in nc.m.functions[0].allocations:
        if not isinstance(alloc, mybir.MemoryLocationSet):
            continue
        name = alloc.memorylocations[0].name
        if alloc.kind == "ExternalInput":
            if name != partition_name:
                in_names.append(name)
        elif alloc.kind == "ExternalOutput":
            shape = tuple(alloc.tensor_shape)
            dtype = mybir.dt.np(alloc.dtype)
            out_names.append(name)
            out_avals.append(jax.core.ShapedArray(shape, dtype))
            zero_shapes.append((shape, dtype))

    n_params = len(in_names)
    n_outs = len(out_avals)
    all_in_names = list(in_names) + list(out_names)
    if partition_name is not None:
        all_in_names.append(partition_name)

    def _body(*args):
        operands = list(args)
        if partition_name is not None:
            operands.append(partition_id_tensor())
        return tuple(_bass_exec_p.bind(
            *operands,
            out_avals=tuple(out_avals),
            in_names=tuple(all_in_names),
            out_names=tuple(out_names),
            lowering_input_output_aliases=(),
            sim_require_finite=False,
            sim_require_nnan=False,
            nc=nc,
        ))

    donate = tuple(range(n_params, n_params + n_outs))
    devices = jax.devices()[:n_cores]
    mesh = Mesh(np.asarray(devices), ("core",))
    in_specs = (PartitionSpec("core"),) * (n_params + n_outs)
    out_specs = (PartitionSpec("core"),) * n_outs
    jf = jax.jit(
        shard_map(_body, mesh=mesh, in_specs=in_specs, out_specs=out_specs,
                  check_rep=False),
        donate_argnums=donate, keep_unused=True)

    def run(in_maps):
        concat_in = [
            np.concatenate([np.asarray(in_maps[c][n]) for c in range(n_cores)], axis=0)
            for n in in_names
        ]
        concat_zeros = [np.zeros((n_cores * s[0], *s[1:]), d) for (s, d) in zero_shapes]
        out_arrs = jf(*concat_in, *concat_zeros)
        jax.block_until_ready(out_arrs)
        return [
            {n: np.asarray(out_arrs[i]).reshape(n_cores, *out_avals[i].shape)[c]
             for i, n in enumerate(out_names)}
            for c in range(n_cores)
        ]

    return run, jf, in_names, zero_shapes


_CACHE = {}
_NC_CACHE = {}


def _get_runner(n_views, Npx):
    key = (n_views, Npx)
    if key not in _CACHE:
        nc = _build_core_kernel(n_views, Npx)
        _NC_CACHE[key] = nc
        _CACHE[key] = _make_runner(nc, N_CORES)
    return _CACHE[key]


def kernel(grids, grid_xy, rgb, idx):
    grids = np.ascontiguousarray(np.asarray(grids, dtype=np.float32))
    grid_xy = np.asarray(grid_xy, dtype=np.float32).astype(np.float16)
    rgb = np.asarray(rgb, dtype=np.float32).astype(np.float16)
    idx = np.asarray(idx)
    N, Hi, Wi, _ = rgb.shape
    Npx = Hi * Wi
    V = N // N_CORES
    g = np.take(grids, idx, axis=0)  # (N, 12, 8, 16, 16)

    run, _, _, _ = _get_runner(V, Npx)
    bsel = _bsel_matrices()
    in_maps = [{
        "grids": g[c * V:(c + 1) * V].reshape(V, 12, -1),
        "gxy": grid_xy[c * V:(c + 1) * V].reshape(V, Npx, 2),
        "rgb": rgb[c * V:(c + 1) * V].reshape(V, Npx, 3),
        "bsel": bsel,
    } for c in range(N_CORES)]
    results = run(in_maps)
    out = np.stack([r["out"] for r in results], axis=0)  # (8, V, Npx, 3) bf16
    return out.reshape(N, Hi, Wi, 3).astype(np.float32)

